# revision 1
# baseline (speedup 1.0000x reference)
"""ExpertPreferredRouter on 8 TRN2 NeuronCores.

Structure:
  - Host: transpose x shards (batch b = core%4, half h = core//4) to [D, H].
  - Phase A (device): logitsT = W @ x_half.T via PE (fp32), softmax over the
    expert (partition) axis -> probsT [64, 2048] per core.
  - Init: per-half top-64 extraction (max/match_replace), pair AllGather of
    (probsT half, candidates) -> full rows r [64, 4096] + merged init
    threshold t0 = exact 64th largest per row.
  - Phase C: damped-rank Jacobi waves on thresholds t_j: per wave, PE applies
    the cross-expert steal mask (strict-upper-triangular matmul on the
    selection mask), fused compare+accumulate gives per-row counts, and the
    threshold descends by up to 16 ranks via DVE max8 candidates.
    Fixpoint == exact greedy expert-preferred assignment.
  - Phase D: disjoint final selection mask -> M (priority matmul) and
    M_probs (masked prob sum matmul).
"""
import os
import sys
import types

import numpy as np

B, N, D, E = 4, 4096, 4096, 64
H = N // 2            # tokens per core (half a batch)
NCORES = 8
WAVES = 22            # numpy raw-rule convergence: 18-19 (dmax=8); margin
DMAX = 8
BIGSEL = float(2.0 ** 100)

TRACE = False         # set True (e.g. by test.py) to capture NTFF timing
LAST_EXEC_NS = None

_cache = {}


def _install_ntff_hook():
    if "antenv.axon_hooks" in sys.modules:
        return
    mod = types.ModuleType("antenv.axon_hooks")
    state = {"hook": None}
    mod.set_axon_ntff_profile_hook = lambda h: state.__setitem__("hook", h)
    mod.get_axon_ntff_profile_hook = lambda: state["hook"]
    sys.modules["antenv.axon_hooks"] = mod
    try:
        import antenv
        antenv.axon_hooks = mod
    except ImportError:
        pass
    try:
        from trn_agent_boot.trn_boot import _ntff_profile_via_ctypes
        mod.set_axon_ntff_profile_hook(
            _ntff_profile_via_ctypes("/opt/axon/libaxon_pjrt.so")
        )
    except Exception:
        pass


def _build_program():
    import concourse.bacc as bacc
    import concourse.mybir as mybir
    from concourse.tile import TileContext
    from concourse.masks import make_identity

    f32 = mybir.dt.float32
    bf16 = mybir.dt.bfloat16
    i32 = mybir.dt.int32
    Alu = mybir.AluOpType

    nc = bacc.Bacc("TRN2", target_bir_lowering=False, num_devices=NCORES)

    xt = nc.dram_tensor("xt", [D, H], f32, kind="ExternalInput")
    wt = nc.dram_tensor("wt", [D, E], f32, kind="ExternalInput")
    mo = nc.dram_tensor("mo", [1, N], f32, kind="ExternalOutput")
    po = nc.dram_tensor("po", [1, N], f32, kind="ExternalOutput")
    co = nc.dram_tensor("co", [E, 1], f32, kind="ExternalOutput")
    DEBUG = bool(int(os.environ.get("KDEBUG", "0")))
    if DEBUG:
        ro = nc.dram_tensor("ro", [E, N], f32, kind="ExternalOutput")
        t0o = nc.dram_tensor("t0o", [E, 1], f32, kind="ExternalOutput")
        cno = nc.dram_tensor("cno", [E, 128], f32, kind="ExternalOutput")

    with TileContext(nc) as tc:
        with (
            tc.tile_pool(name="persist", bufs=1) as pp,
            tc.tile_pool(name="work", bufs=1) as wp,
            tc.tile_pool(name="stream", bufs=3) as sp,
            tc.tile_pool(name="small", bufs=2) as smp,
            tc.tile_pool(name="dram", bufs=1, space="DRAM") as dp,
        ):
            # ---------------- Phase A: matmul ----------------
            # wt_sb[p, dc*64+e] = wt[dc*128+p, e]
            wt_sb = pp.tile([128, 32 * E], f32, tag="wt")
            nc.sync.dma_start(
                wt_sb[:].rearrange("p (c e) -> p c e", e=E),
                wt[:].rearrange("(c p) e -> p c e", p=128),
            )

            probsT = pp.tile([E, H], f32, tag="probsT")
            with tc.tile_pool(name="plog", bufs=1, space="PSUM") as plog_pool:
                psumL = plog_pool.tile([E, H], f32, tag="plog")
                for dc in range(32):
                    xchunk = sp.tile([128, H], f32, tag="xchunk")
                    nc.sync.dma_start(xchunk[:], xt[dc * 128 : (dc + 1) * 128, :])
                    for nt in range(4):
                        sl = slice(nt * 512, (nt + 1) * 512)
                        nc.tensor.matmul(
                            psumL[:, sl],
                            wt_sb[:, dc * E : (dc + 1) * E],
                            xchunk[:, sl],
                            start=(dc == 0),
                            stop=(dc == 31),
                        )
                # softmax over experts (partition axis), no max-subtraction
                # (|logits| <~ 5 so exp is safe in fp32)
                expT = wp.tile([E, H], f32, tag="expT")
                nc.scalar.activation(
                    expT[:], psumL[:], mybir.ActivationFunctionType.Exp
                )

            ones64 = pp.tile([E, 1], f32, tag="ones64")
            nc.vector.memset(ones64[:], 1.0)
            with tc.tile_pool(name="pz", bufs=1, space="PSUM") as pz_pool:
                pz = pz_pool.tile([1, H], f32, tag="pz")
                for ch in range(4):
                    sl = slice(ch * 512, (ch + 1) * 512)
                    nc.tensor.matmul(
                        pz[:, sl], ones64[:], expT[:, sl], start=True, stop=True
                    )
                zrow = wp.tile([1, H], f32, tag="zrow")
                nc.vector.reciprocal(zrow[:], pz[:])

            one1 = pp.tile([1, E], f32, tag="one1")
            nc.vector.memset(one1[:], 1.0)
            with tc.tile_pool(name="pw", bufs=1, space="PSUM") as pw_pool:
                pwb = pw_pool.tile([E, H], f32, tag="pwb")
                for ch in range(4):
                    sl = slice(ch * 512, (ch + 1) * 512)
                    nc.tensor.matmul(
                        pwb[:, sl], one1[:], zrow[:, sl], start=True, stop=True
                    )
                nc.vector.tensor_mul(probsT[:], expT[:], pwb[:])

            # ---------------- Init: per-half top-64 ----------------
            candL = wp.tile([E, 64], f32, tag="candL")
            wrkA = wp.tile([E, H], f32, tag="wrkA")
            wrkB = wp.tile([E, H], f32, tag="wrkB")
            nc.vector.tensor_copy(wrkA[:], probsT[:])
            cur, nxt = wrkA, wrkB
            for rnd in range(8):
                m8 = smp.tile([E, 8], f32, tag="m8")
                nc.vector.max(m8[:], cur[:])
                nc.vector.tensor_copy(candL[:, rnd * 8 : rnd * 8 + 8], m8[:])
                if rnd < 7:
                    nc.vector.match_replace(
                        out=nxt[:], in_to_replace=m8[:], in_values=cur[:],
                        imm_value=-1e38,
                    )
                    cur, nxt = nxt, cur

            # ---------------- AllGather pair {c, c+4} ----------------
            agin = dp.tile([E, H + 64], f32)
            agout = dp.tile([2, E, H + 64], f32)
            nc.sync.dma_start(agin[:, :H], probsT[:])
            nc.sync.dma_start(agin[:, H:], candL[:])
            nc.gpsimd.collective_compute(
                "AllGather",
                mybir.AluOpType.bypass,
                replica_groups=[[0, 4], [1, 5], [2, 6], [3, 7]],
                ins=[agin.opt()],
                outs=[agout.opt()],
            )
            r_sb = pp.tile([E, N], f32, tag="r")
            candAB = wp.tile([E, 128], f32, tag="candAB")
            for h in range(2):
                nc.sync.dma_start(r_sb[:, h * H : (h + 1) * H], agout[h, :, :H])
                nc.sync.dma_start(candAB[:, h * 64 : (h + 1) * 64], agout[h, :, H:])

            # ---------------- t0 = 64th largest of merged halves ----------------
            # union-kth: t0 = max_{i+j=64} min(A_i, B_j), A_0 = B_0 = +inf
            apad = wp.tile([E, 65], f32, tag="apad")
            brev = wp.tile([E, 65], f32, tag="brev")
            nc.vector.memset(apad[:, :1], 1e38)
            nc.vector.tensor_copy(apad[:, 1:], candAB[:, :64])
            nc.vector.memset(brev[:, 64:], 1e38)
            # brev[:, k] = B_(64-k) = candB reversed (negative-step read AP)
            nc.vector.tensor_copy(brev[:, :64], candAB[:, 64:][:, ::-1])
            tmin = wp.tile([E, 65], f32, tag="tmin")
            nc.vector.tensor_tensor(
                tmin[:], apad[:], brev[:], op=Alu.min
            )
            t_vec = pp.tile([E, 1], f32, tag="t")
            nc.vector.tensor_reduce(
                t_vec[:], tmin[:], axis=mybir.AxisListType.X, op=Alu.max
            )

            if DEBUG:
                nc.sync.dma_start(ro[:], r_sb[:])
                nc.sync.dma_start(t0o[:], t_vec[:])
                nc.sync.dma_start(cno[:], candAB[:])

            # ---------------- wave constants ----------------
            ident64 = pp.tile([E, E], f32, tag="ident64")
            make_identity(nc, ident64)
            # ustrict[j', j] = 1 if j' > j (bf16 weights for the msk matmul)
            iota_col = pp.tile([E, E], i32, tag="iotacol")
            nc.gpsimd.iota(iota_col[:], pattern=[[1, E]], base=0, channel_multiplier=0)
            ustrict = pp.tile([E, E], bf16, tag="ustrict")
            # row p: 1 where col < p  <=>  iota_col[p, c] = c < p
            pidx = pp.tile([E, 1], i32, tag="pidx")
            nc.gpsimd.iota(pidx[:], pattern=[[0, 1]], base=0, channel_multiplier=1)
            pidx_f = pp.tile([E, 1], f32, tag="pidxf")
            nc.vector.tensor_copy(pidx_f[:], pidx[:])
            iota_f = pp.tile([E, E], f32, tag="iotaf")
            nc.vector.tensor_copy(iota_f[:], iota_col[:])
            nc.vector.tensor_scalar(
                ustrict[:], iota_f[:], pidx_f[:], -BIGSEL, op0=Alu.is_lt, op1=Alu.mult
            )
            iota16 = pp.tile([E, 16], f32, tag="iota16")
            i16 = pp.tile([E, 16], i32, tag="i16")
            nc.gpsimd.iota(i16[:], pattern=[[1, 16]], base=0, channel_multiplier=0)
            nc.vector.tensor_copy(iota16[:], i16[:])
            jvec_bf = pp.tile([E, 1], bf16, tag="jvecbf")
            nc.vector.tensor_copy(jvec_bf[:], pidx_f[:])
            ones_bf = pp.tile([E, 1], bf16, tag="onesbf")
            nc.vector.memset(ones_bf[:], 1.0)

            msk = pp.tile([E, N], bf16, tag="msk1", name="msk_init")
            nc.vector.memset(msk[:], 0.0)
            w_sb = wp.tile([E, N], f32, tag="wsb")
            w2_sb = wp.tile([E, N], f32, tag="w2sb")
            cand16 = smp.tile([E, 16], f32, tag="cand16")
            cntp = smp.tile([E, 8], f32, tag="cntp")

            NCH = 8
            CW = N // NCH

            # ---------------- Phase C: waves ----------------
            # msk fed to the steal matmul uses the RAW rule (r >= t): steal
            # signals propagate in one hop, converging ~2x faster than the
            # masked rule. Counts/candidates use the masked values (pm).
            sel_m = wp.tile([E, N], bf16, tag="selm")
            for wv in range(WAVES + 1):
                last = wv == WAVES
                cand8 = smp.tile([E, 8], f32, tag="cand8")
                cntp = smp.tile([E, 8], f32, tag="cntp")
                if not last:
                    # raw-rule mask for next wave's steal matmul
                    mskn = pp.tile([E, N], bf16, tag=f"msk{wv % 2}", name=f"mskn{wv}")
                    nc.vector.tensor_scalar(
                        mskn[:], r_sb[:], t_vec[:], None, op0=Alu.is_ge
                    )
                else:
                    msk01 = pp.tile([E, N], bf16, tag="msk01")
                with tc.tile_pool(name=f"pmw{wv}", bufs=4, space="PSUM") as pmp:
                    for ch in range(NCH):
                        sl = slice(ch * CW, (ch + 1) * CW)
                        pm = pmp.tile([E, CW], f32, tag="pm")
                        nc.tensor.matmul(
                            pm[:], ident64[:], r_sb[:, sl], start=True, stop=False
                        )
                        nc.tensor.matmul(
                            pm[:], ustrict[:], msk[:, sl], start=False, stop=True
                        )
                        dst = msk01 if last else sel_m
                        nc.vector.tensor_scalar(
                            dst[:, sl], pm[:], t_vec[:], None,
                            op0=Alu.is_ge, op1=Alu.add,
                            accum_out=cntp[:, ch : ch + 1],
                        )
                        if not last:
                            # w = masked row with >=t values knocked out
                            nc.vector.scalar_tensor_tensor(
                                w_sb[:, sl], sel_m[:, sl], -BIGSEL, pm[:],
                                op0=Alu.mult, op1=Alu.add,
                            )
                if last:
                    cnt = smp.tile([E, 1], f32, tag="cnt")
                    nc.vector.tensor_reduce(
                        cnt[:], cntp[:], axis=mybir.AxisListType.X, op=Alu.add
                    )
                    nc.sync.dma_start(co[:], cnt[:])
                    msk = msk01
                    break

                cnt = smp.tile([E, 1], f32, tag="cnt")
                nc.vector.tensor_reduce(
                    cnt[:], cntp[:], axis=mybir.AxisListType.X, op=Alu.add
                )
                # d = clamp(64 - cnt, 0, DMAX); dm1 = d - 1
                dm1 = smp.tile([E, 1], f32, tag="dm1")
                nc.vector.tensor_scalar(
                    dm1[:], cnt[:], -1.0, 64.0,
                    op0=Alu.mult, op1=Alu.add,
                )
                nc.vector.tensor_scalar_min(dm1[:], dm1[:], float(DMAX))
                # dm1 = d - 1 (=-1 when d=0 -> onehot all zero)
                nc.vector.tensor_scalar_add(dm1[:], dm1[:], -1.0)

                # candidates: top-8 of w rows
                nc.vector.max(cand8[:], w_sb[:])

                # t_new = cand8[d-1]  (keep t when d == 0)
                oh = smp.tile([E, 8], f32, tag="oh")
                nc.vector.tensor_scalar(
                    oh[:], iota16[:, :8], dm1[:], None, op0=Alu.is_equal
                )
                tsel = smp.tile([E, 8], f32, tag="tsel")
                nc.vector.tensor_mul(tsel[:], cand8[:], oh[:])
                tnew = smp.tile([E, 1], f32, tag="tnew")
                nc.vector.tensor_reduce(
                    tnew[:], tsel[:], axis=mybir.AxisListType.X, op=Alu.add
                )
                z = smp.tile([E, 1], f32, tag="z")
                nc.vector.tensor_scalar(
                    z[:], dm1[:], -1.0, None, op0=Alu.is_equal
                )
                zk = smp.tile([E, 1], f32, tag="zk")
                nc.vector.tensor_mul(zk[:], z[:], t_vec[:])
                t_vec = pp.tile([E, 1], f32, tag=f"t{wv % 2}", name=f"tvec{wv}")
                nc.vector.tensor_add(t_vec[:], tnew[:], zk[:])
                msk = mskn

            # ---------------- Phase D: outputs ----------------
            # M = sum_j j * msk01[j, n]  (disjoint selection)
            psel = wp.tile([E, N], f32, tag="psel")
            nc.vector.tensor_mul(psel[:], r_sb[:], msk[:])
            mo_sb = wp.tile([1, N], f32, tag="mo")
            po_sb = wp.tile([1, N], f32, tag="po")
            with tc.tile_pool(name="pout", bufs=4, space="PSUM") as pop:
                for ch in range(8):
                    sl = slice(ch * 512, (ch + 1) * 512)
                    pmm = pop.tile([1, 512], f32, tag="pmm")
                    nc.tensor.matmul(
                        pmm[:], jvec_bf[:], msk[:, sl], start=True, stop=True
                    )
                    nc.vector.tensor_copy(mo_sb[:, sl], pmm[:])
                    ppp = pop.tile([1, 512], f32, tag="ppp")
                    nc.tensor.matmul(
                        ppp[:], ones64[:], psel[:, sl], start=True, stop=True
                    )
                    nc.vector.tensor_copy(po_sb[:, sl], ppp[:])
            nc.sync.dma_start(mo[:], mo_sb[:])
            nc.sync.dma_start(po[:], po_sb[:])

    nc.compile()
    return nc


def kernel(x, W, c):
    global LAST_EXEC_NS
    from concourse import bass_utils

    x = np.asarray(x, dtype=np.float32)
    W = np.asarray(W, dtype=np.float32)

    if "nc" not in _cache:
        _cache["nc"] = _build_program()
    nc = _cache["nc"]

    wt_host = np.ascontiguousarray(W.T)  # [D, E]
    in_maps = []
    for core in range(NCORES):
        b, h = core % B, core // B
        xt_host = np.ascontiguousarray(x[b, h * H : (h + 1) * H, :].T)  # [D, H]
        in_maps.append({"xt": xt_host, "wt": wt_host})

    trace = TRACE
    if trace:
        _install_ntff_hook()
    res = bass_utils.run_bass_kernel_spmd(
        nc, in_maps, core_ids=list(range(NCORES)), trace=trace
    )
    LAST_EXEC_NS = res.exec_time_ns

    M = np.zeros((B, N), dtype=np.int32)
    P = np.zeros((B, N), dtype=np.float32)
    for b in range(B):
        out = res.results[b]
        cnt = out["co"][:, 0]
        if not np.allclose(cnt, 64.0):
            print(f"[kernel] WARNING: batch {b} expert counts != 64: "
                  f"min={cnt.min()} max={cnt.max()}", file=sys.stderr)
        M[b] = np.rint(out["mo"][0]).astype(np.int32)
        P[b] = out["po"][0].astype(np.float32)
    return M, P



# revision 7
# speedup vs baseline: 1.2851x; 1.2851x over previous
"""ExpertPreferredRouter on 8 TRN2 NeuronCores (folded wave redesign).

Per core (batch b = core%4, half h = core//4):
  Phase A: stream x in 8 chunks of 256 tokens; logitsT = W @ x.T on PE
    (fp32r by default), per-chunk softmax (exp on Act, z via PE, recip on
    DVE), per-512-token-group top-8 candidates for the descent-safe t0
    bound (t0 = max over 8 groups of the group 8th-largest >= true 64th).
  Phase B: 4 chunked pair AllGathers of probs (512 tokens each) pipelined
    under phase A; result assembled into the folded wave array
    r2[j + 64*half, n] (128 partitions x 2048 tokens).
  Phase C: damped threshold waves (DMAX=16/wave). Act engine computes the
    raw mask as Sign(r - t); PE turns it into steal counts
    P = 2^20 * sum_{j'>j}(sign+1) via a strict-upper bf16 matmul plus a
    constant row (P == 0 iff unstolen, fp32-exact in PSUM); DVE does
    pm = r - P, sel/count (is_ge + accum), candidate knockout, and
    per-partition top-16 (max8 / match_replace / max8). Cross-partition
    (half) candidate+count merge via a small fp32 permutation matmul.
    Fixpoint == exact greedy expert-preferred assignment.
  Phase D: disjoint final mask -> M (priority matmul) and M_probs
    (masked prob sum matmul), both halves, full-row outputs.
"""
import os
import sys
import types

import numpy as np

B, N, D, E = 4, 4096, 4096, 64
H = N // 2            # tokens per core (half a batch)
HF = N // 2           # folded free size (tokens per partition row)
NCORES = 8
NT = 8                # phase A chunks per core
CT = H // NT          # 256 tokens per phase-A chunk
BIGW = float(2.0 ** 20)
NWAVES = 14           # numpy convergence (dmax16 + chunk8 t0): 12; +2 margin
DMAX = 16

TRACE = False
LAST_EXEC_NS = None

_cache = {}


def _install_ntff_hook():
    if "antenv.axon_hooks" in sys.modules:
        return
    mod = types.ModuleType("antenv.axon_hooks")
    state = {"hook": None}
    mod.set_axon_ntff_profile_hook = lambda h: state.__setitem__("hook", h)
    mod.get_axon_ntff_profile_hook = lambda: state["hook"]
    sys.modules["antenv.axon_hooks"] = mod
    try:
        import antenv
        antenv.axon_hooks = mod
    except ImportError:
        pass
    try:
        from trn_agent_boot.trn_boot import _ntff_profile_via_ctypes
        mod.set_axon_ntff_profile_hook(
            _ntff_profile_via_ctypes("/opt/axon/libaxon_pjrt.so")
        )
    except Exception:
        pass


def _build_program():
    import concourse.bacc as bacc
    import concourse.mybir as mybir
    from concourse.tile import TileContext

    f32 = mybir.dt.float32
    f32r = mybir.dt.float32r
    bf16 = mybir.dt.bfloat16
    i32 = mybir.dt.int32
    Alu = mybir.AluOpType
    Act = mybir.ActivationFunctionType

    USE_F32R = bool(int(os.environ.get("KF32R", "1")))
    DEBUG = bool(int(os.environ.get("KDEBUG", "0")))

    nc = bacc.Bacc("TRN2", target_bir_lowering=False, num_devices=NCORES)

    xt = nc.dram_tensor("xt", [D, H], f32, kind="ExternalInput")
    wt = nc.dram_tensor("wt", [D, E], f32, kind="ExternalInput")
    mo = nc.dram_tensor("mo", [1, N], f32, kind="ExternalOutput")
    po = nc.dram_tensor("po", [1, N], f32, kind="ExternalOutput")
    co = nc.dram_tensor("co", [128, 2], f32, kind="ExternalOutput")
    if DEBUG:
        ro = nc.dram_tensor("ro", [128, HF], f32, kind="ExternalOutput")
        t0o = nc.dram_tensor("t0o", [128, 1], f32, kind="ExternalOutput")
        tfo = nc.dram_tensor("tfo", [128, 1], f32, kind="ExternalOutput")

    with TileContext(nc) as tc:
        with (
            tc.tile_pool(name="persist", bufs=1) as pp,
            tc.tile_pool(name="work", bufs=1) as wp,
            tc.tile_pool(name="xstream", bufs=2) as xp,
            tc.tile_pool(name="achunk", bufs=2) as ap_,
            tc.tile_pool(name="small", bufs=2) as smp,
            tc.tile_pool(name="dram", bufs=1, space="DRAM") as dp,
        ):
            # ================= constants =================
            iota128 = pp.tile([128, 128], i32, tag="iota128")
            nc.gpsimd.iota(iota128[:], pattern=[[1, 128]], base=0,
                           channel_multiplier=0)
            pidx = pp.tile([128, 1], i32, tag="pidx")
            nc.gpsimd.iota(pidx[:], pattern=[[0, 1]], base=0,
                           channel_multiplier=1)
            col_f = pp.tile([128, 128], f32, tag="colf")
            nc.vector.tensor_copy(col_f[:], iota128[:])
            pidx_f = pp.tile([128, 1], f32, tag="pidxf")
            nc.vector.tensor_copy(pidx_f[:], pidx[:])

            colhi = wp.tile([128, 128], f32, tag="colhi")
            nc.vector.tensor_scalar(colhi[:], col_f[:], 64.0, None, op0=Alu.is_ge)
            colmod = pp.tile([128, 128], f32, tag="colmod")
            nc.vector.scalar_tensor_tensor(
                colmod[:], colhi[:], -64.0, col_f[:], op0=Alu.mult, op1=Alu.add)
            rowhi = pp.tile([128, 1], f32, tag="rowhi")
            nc.vector.tensor_scalar(rowhi[:], pidx_f[:], 64.0, None, op0=Alu.is_ge)
            rowmod = pp.tile([128, 1], f32, tag="rowmod")
            nc.vector.scalar_tensor_tensor(
                rowmod[:], rowhi[:], -64.0, pidx_f[:], op0=Alu.mult, op1=Alu.add)

            # Ubd[c, m] = 2^20 if same 64-block and (m%64) < (c%64)
            sameblk = wp.tile([128, 128], f32, tag="sameblk")
            nc.vector.tensor_scalar(
                sameblk[:], colhi[:], rowhi[:], None, op0=Alu.is_equal)
            strict = wp.tile([128, 128], f32, tag="strict")
            nc.vector.tensor_scalar(
                strict[:], colmod[:], rowmod[:], None, op0=Alu.is_lt)
            ubd_f = wp.tile([128, 128], f32, tag="ubdf")
            nc.vector.tensor_mul(ubd_f[:], strict[:], sameblk[:])
            Ubd = pp.tile([128, 128], bf16, tag="Ubd")
            nc.vector.tensor_scalar(Ubd[:], ubd_f[:], BIGW, None, op0=Alu.mult)

            # cvec_w[0, m] = (63 - m%64) * 2^20
            cvec_w = pp.tile([1, 128], bf16, tag="cvecw")
            cv_f = smp.tile([1, 128], f32, tag="cvf")
            nc.vector.tensor_scalar(
                cv_f[:], colmod[0:1, :], -1.0, 63.0, op0=Alu.mult, op1=Alu.add)
            nc.vector.tensor_scalar(cvec_w[:], cv_f[:], BIGW, None, op0=Alu.mult)

            # perm[c, m] = 1 iff c == m + 64  (move partitions 64.. -> 0..)
            iota64c = pp.tile([128, 64], i32, tag="iota64c")
            nc.gpsimd.iota(iota64c[:], pattern=[[1, 64]], base=0,
                           channel_multiplier=0)
            iota64f = wp.tile([128, 64], f32, tag="iota64f")
            nc.vector.tensor_copy(iota64f[:], iota64c[:])
            pidxm64 = smp.tile([128, 1], f32, tag="pidxm64")
            nc.vector.tensor_scalar_add(pidxm64[:], pidx_f[:], -64.0)
            perm = pp.tile([128, 64], f32, tag="perm")
            nc.vector.tensor_scalar(
                perm[:], iota64f[:], pidxm64[:], None, op0=Alu.is_equal)

            # dup[c, m] = 1 iff (m % 64) == c  (broadcast t64 -> both halves)
            dup = pp.tile([64, 128], f32, tag="dup")
            nc.vector.tensor_scalar(
                dup[:], colmod[0:64, :], pidx_f[0:64], None, op0=Alu.is_equal)

            # output weight vectors
            notrowhi = smp.tile([128, 1], f32, tag="notrowhi")
            nc.vector.tensor_scalar(
                notrowhi[:], rowhi[:], -1.0, 1.0, op0=Alu.mult, op1=Alu.add)
            jw = []
            pw = []
            for hh, hvec in ((0, notrowhi), (1, rowhi)):
                jwh = pp.tile([128, 1], bf16, tag=f"jw{hh}")
                nc.vector.tensor_mul(jwh[:], rowmod[:], hvec[:])
                jw.append(jwh)
                pwh = pp.tile([128, 1], bf16, tag=f"pw{hh}")
                nc.vector.tensor_copy(pwh[:], hvec[:])
                pw.append(pwh)

            iota17 = pp.tile([64, 17], i32, tag="iota17i")
            nc.gpsimd.iota(iota17[:], pattern=[[1, 17]], base=0,
                           channel_multiplier=0)
            iota17m1 = pp.tile([64, 17], f32, tag="iota17m1")
            nc.vector.tensor_copy(iota17m1[:], iota17[:])
            nc.vector.tensor_scalar_add(iota17m1[:], iota17m1[:], -1.0)

            ones64 = pp.tile([64, 1], f32, tag="ones64")
            nc.vector.memset(ones64[:], 1.0)
            one1 = pp.tile([1, E], f32, tag="one1")
            nc.vector.memset(one1[:], 1.0)
            onesrow = pp.tile([1, 512], bf16, tag="onesrow")
            nc.vector.memset(onesrow[:], 1.0)

            # ================= Phase A =================
            # wt_sb[p, dc*64+e] = wt[dc*128+p, e]
            wt_sb = pp.tile([128, 32 * E], f32, tag="wt")
            nc.sync.dma_start(
                wt_sb[:].rearrange("p (c e) -> p c e", e=E),
                wt[:].rearrange("(c p) e -> p c e", p=128),
            )

            r2 = pp.tile([128, HF], f32, tag="r2")
            pstage = wp.tile([64, H], f32, tag="pstage")
            my4 = smp.tile([64, 8], f32, tag="my4")

            agp_in = [dp.tile([64, 512], f32, name=f"agp_in{g}")
                      for g in range(4)]
            agc_in = dp.tile([64, 4], f32)
            agp_out = [dp.tile([2, 64, 512], f32, name=f"agp_out{g}")
                      for g in range(4)]
            agc_out = dp.tile([2, 64, 4], f32)

            with tc.tile_pool(name="paL", bufs=1, space="PSUM") as paL, \
                 tc.tile_pool(name="paZ", bufs=2, space="PSUM") as paZ:
                psumL = paL.tile([64, H], f32, tag="plog")
                for k in range(NT):
                    sl = slice(k * CT, (k + 1) * CT)
                    xchunk = xp.tile([128, 32 * CT], f32, tag="xchunk")
                    nc.sync.dma_start(
                        xchunk[:].rearrange("p (c n) -> p c n", n=CT),
                        xt[:, sl].rearrange("(c p) n -> p c n", p=128),
                    )
                    for dc in range(32):
                        wslice = wt_sb[:, dc * E:(dc + 1) * E]
                        xslice = xchunk[:, dc * CT:(dc + 1) * CT]
                        if USE_F32R:
                            wslice = wslice.bitcast(f32r)
                            xslice = xslice.bitcast(f32r)
                        nc.tensor.matmul(
                            psumL[:, sl], wslice, xslice,
                            start=(dc == 0), stop=(dc == 31),
                        )
                    # softmax chunk: no max-subtraction (|logits| small)
                    expT = ap_.tile([64, CT], f32, tag="expT")
                    nc.scalar.activation(expT[:], psumL[:, sl], Act.Exp)
                    pz = paZ.tile([1, CT], f32, tag="pz")
                    nc.tensor.matmul(
                        pz[:], ones64[:], expT[:], start=True, stop=True)
                    zrow = ap_.tile([1, CT], f32, tag="zrow")
                    nc.vector.reciprocal(zrow[:], pz[:])
                    pwb = paZ.tile([64, CT], f32, tag="pwb")
                    nc.tensor.matmul(
                        pwb[:], one1[:], zrow[:], start=True, stop=True)
                    nc.vector.tensor_mul(pstage[:, sl], expT[:], pwb[:])
                    if k % 2 == 1:
                        g = k // 2
                        gsl = slice(g * 512, (g + 1) * 512)
                        c8 = smp.tile([64, 8], f32, tag="c8")
                        nc.vector.max(c8[:], pstage[:, gsl])
                        nc.vector.tensor_copy(my4[:, g:g + 1], c8[:, 7:8])
                        nc.sync.dma_start(agp_in[g][:], pstage[:, gsl])
                        nc.gpsimd.collective_compute(
                            "AllGather",
                            Alu.bypass,
                            replica_groups=[[0, 4], [1, 5], [2, 6], [3, 7]],
                            ins=[agp_in[g].opt()],
                            outs=[agp_out[g].opt()],
                        )
                        nc.sync.dma_start(r2[0:64, gsl], agp_out[g][0])
                        nc.sync.dma_start(r2[64:128, gsl], agp_out[g][1])

            # candidate AllGather -> t0 bound (max over 8 group 8th-largest)
            nc.sync.dma_start(agc_in[:], my4[:, 0:4])
            nc.gpsimd.collective_compute(
                "AllGather",
                Alu.bypass,
                replica_groups=[[0, 4], [1, 5], [2, 6], [3, 7]],
                ins=[agc_in.opt()],
                outs=[agc_out.opt()],
            )
            c8both = smp.tile([64, 8], f32, tag="c8both")
            nc.sync.dma_start(c8both[:, 0:4], agc_out[0])
            nc.sync.dma_start(c8both[:, 4:8], agc_out[1])

            t128 = pp.tile([128, 1], f32, tag="t128")
            nc.vector.tensor_reduce(
                t128[0:64, :], c8both[:], axis=mybir.AxisListType.X, op=Alu.max)
            negt = pp.tile([128, 1], f32, tag="negt")

            with tc.tile_pool(name="ptb", bufs=1, space="PSUM") as ptb:
                t128p = ptb.tile([128, 1], f32, tag="t128p")
                nc.tensor.matmul(
                    t128p[:], dup[:], t128[0:64, :], start=True, stop=True)
                nc.vector.tensor_copy(t128[:], t128p[:])
            nc.vector.tensor_scalar_mul(negt[:], t128[:], -1.0)

            if DEBUG:
                nc.sync.dma_start(ro[:], r2[:])
                nc.sync.dma_start(t0o[:], t128[:])

            # ================= Phase C: waves =================
            msk = pp.tile([128, HF], bf16, tag="msk")
            pm = pp.tile([128, HF], f32, tag="pm")
            sel = pp.tile([128, HF], bf16, tag="sel")
            w_sb = pp.tile([128, HF], f32, tag="wsb")
            w_tmp = pp.tile([128, HF], f32, tag="wtmp")
            candpool = pp.tile([128, 17], f32, tag="candpool")
            mg = pp.tile([64, 32], f32, tag="mg")
            mgt = pp.tile([64, 32], f32, tag="mgt")
            candsel = pp.tile([64, 17], f32, tag="candsel")

            for wv in range(NWAVES + 1):
                last = wv == NWAVES

                # raw mask: sign(r - t) on the Act engine
                for c in range(2):
                    csl = slice(c * 1024, (c + 1) * 1024)
                    nc.scalar.activation(
                        msk[:, csl], r2[:, csl], Act.Sign, bias=negt[:])

                with tc.tile_pool(name=f"pw{wv}", bufs=1, space="PSUM") as pwp:
                    Pp = pwp.tile([128, HF], f32, tag="Pp")
                    for c in range(4):
                        csl = slice(c * 512, (c + 1) * 512)
                        nc.tensor.matmul(
                            Pp[:, csl], Ubd[:], msk[:, csl],
                            start=True, stop=False)
                    for c in range(4):
                        csl = slice(c * 512, (c + 1) * 512)
                        nc.tensor.matmul(
                            Pp[:, csl], cvec_w[:], onesrow[:],
                            start=False, stop=True)
                    cnt2 = smp.tile([128, 2], f32, tag="cnt2")
                    for c in range(2):
                        csl = slice(c * 1024, (c + 1) * 1024)
                        nc.vector.scalar_tensor_tensor(
                            pm[:, csl], Pp[:, csl], -1.0, r2[:, csl],
                            op0=Alu.mult, op1=Alu.add)
                        nc.vector.tensor_scalar(
                            sel[:, csl], pm[:, csl], t128[:], None,
                            op0=Alu.is_ge, op1=Alu.add,
                            accum_out=cnt2[:, c:c + 1],
                        )

                if last:
                    cnt2f = smp.tile([128, 2], f32, tag="cnt2f")
                    nc.vector.tensor_copy(cnt2f[:], cnt2[:])
                    nc.sync.dma_start(co[:], cnt2f[:])
                    break

                for c in range(2):
                    csl = slice(c * 1024, (c + 1) * 1024)
                    nc.vector.scalar_tensor_tensor(
                        w_sb[:, csl], sel[:, csl], -BIGW, pm[:, csl],
                        op0=Alu.mult, op1=Alu.add)

                # per-partition top-16 list into candpool[:, 0:16]
                nc.vector.max(candpool[:, 0:8], w_sb[:])
                nc.vector.match_replace(
                    out=w_tmp[:], in_to_replace=candpool[:, 0:8],
                    in_values=w_sb[:], imm_value=-1e30)
                nc.vector.max(candpool[:, 8:16], w_tmp[:])

                cnt1 = smp.tile([128, 1], f32, tag="cnt1")
                nc.vector.tensor_reduce(
                    cnt1[:], cnt2[:], axis=mybir.AxisListType.X, op=Alu.add)
                nc.vector.tensor_copy(candpool[:, 16:17], cnt1[:])

                # move hi-half (partitions 64..127) candidates+count down
                with tc.tile_pool(name=f"pc{wv}", bufs=1, space="PSUM") as pcp:
                    pcand = pcp.tile([64, 17], f32, tag="pcand")
                    nc.tensor.matmul(
                        pcand[:], perm[:], candpool[:], start=True, stop=True)
                    nc.vector.tensor_copy(mg[:, 0:16], candpool[0:64, 0:16])
                    nc.vector.tensor_copy(mg[:, 16:32], pcand[:, 0:16])
                    cntfull = smp.tile([64, 1], f32, tag="cntfull")
                    nc.vector.tensor_tensor(
                        cntfull[:], candpool[0:64, 16:17], pcand[:, 16:17],
                        op=Alu.add)

                # top-16 of the merged 32 candidates -> candsel[:, 1:17]
                nc.vector.tensor_copy(candsel[:, 0:1], t128[0:64, :])
                nc.vector.max(candsel[:, 1:9], mg[:])
                nc.vector.match_replace(
                    out=mgt[:], in_to_replace=candsel[:, 1:9],
                    in_values=mg[:], imm_value=-1e30)
                nc.vector.max(candsel[:, 9:17], mgt[:])

                # dm1 = min(64 - cnt, DMAX) - 1; t_new = candsel[dm1 + 1]
                dm1 = smp.tile([64, 1], f32, tag="dm1")
                nc.vector.tensor_scalar(
                    dm1[:], cntfull[:], -1.0, 64.0, op0=Alu.mult, op1=Alu.add)
                nc.vector.tensor_scalar_min(dm1[:], dm1[:], float(DMAX))
                nc.vector.tensor_scalar_add(dm1[:], dm1[:], -1.0)
                oh = smp.tile([64, 17], f32, tag="oh")
                nc.vector.tensor_scalar(
                    oh[:], iota17m1[:], dm1[:], None, op0=Alu.is_equal)
                tsel = smp.tile([64, 17], f32, tag="tsel")
                nc.vector.tensor_mul(tsel[:], oh[:], candsel[:])
                nc.vector.tensor_reduce(
                    t128[0:64, :], tsel[:],
                    axis=mybir.AxisListType.X, op=Alu.add)

                # broadcast t to both partition halves; negate for Act bias
                with tc.tile_pool(name=f"pt{wv}", bufs=1, space="PSUM") as ptp:
                    t128p = ptp.tile([128, 1], f32, tag="t128p")
                    nc.tensor.matmul(
                        t128p[:], dup[:], t128[0:64, :], start=True, stop=True)
                    nc.vector.tensor_copy(t128[:], t128p[:])
                nc.vector.tensor_scalar_mul(negt[:], t128[:], -1.0)

            if DEBUG:
                nc.sync.dma_start(tfo[:], t128[:])

            # ================= Phase D: outputs =================
            msk01 = sel
            psel = wp.tile([128, HF], bf16, tag="psel")
            nc.vector.tensor_mul(psel[:], r2[:], msk01[:])
            mo_sb = wp.tile([1, N], f32, tag="mo")
            po_sb = wp.tile([1, N], f32, tag="po")
            with tc.tile_pool(name="pout", bufs=4, space="PSUM") as pop:
                for hh in range(2):
                    for ch in range(4):
                        csl = slice(ch * 512, (ch + 1) * 512)
                        osl = slice(hh * HF + ch * 512, hh * HF + (ch + 1) * 512)
                        pmm = pop.tile([1, 512], f32, tag="pmm")
                        nc.tensor.matmul(
                            pmm[:], jw[hh][:], msk01[:, csl],
                            start=True, stop=True)
                        nc.vector.tensor_copy(mo_sb[:, osl], pmm[:])
                        ppp = pop.tile([1, 512], f32, tag="ppp")
                        nc.tensor.matmul(
                            ppp[:], pw[hh][:], psel[:, csl],
                            start=True, stop=True)
                        nc.vector.tensor_copy(po_sb[:, osl], ppp[:])
            nc.sync.dma_start(mo[:], mo_sb[:])
            nc.sync.dma_start(po[:], po_sb[:])

    nc.compile()
    return nc


def kernel(x, W, c):
    global LAST_EXEC_NS
    from concourse import bass_utils

    x = np.asarray(x, dtype=np.float32)
    W = np.asarray(W, dtype=np.float32)

    if "nc" not in _cache:
        _cache["nc"] = _build_program()
    nc = _cache["nc"]

    wt_host = np.ascontiguousarray(W.T)  # [D, E]
    in_maps = []
    for core in range(NCORES):
        b, h = core % B, core // B
        xt_host = np.ascontiguousarray(x[b, h * H:(h + 1) * H, :].T)  # [D, H]
        in_maps.append({"xt": xt_host, "wt": wt_host})

    trace = TRACE
    if trace:
        _install_ntff_hook()
    res = bass_utils.run_bass_kernel_spmd(
        nc, in_maps, core_ids=list(range(NCORES)), trace=trace
    )
    LAST_EXEC_NS = res.exec_time_ns

    M = np.zeros((B, N), dtype=np.int32)
    P = np.zeros((B, N), dtype=np.float32)
    for b in range(B):
        out = res.results[b]
        cnt = out["co"]  # [128, 2]
        cj = cnt[0:64, 0] + cnt[0:64, 1] + cnt[64:128, 0] + cnt[64:128, 1]
        if not np.allclose(cj, 64.0):
            print(f"[kernel] WARNING: batch {b} expert counts != 64: "
                  f"min={cj.min()} max={cj.max()}", file=sys.stderr)
        M[b] = np.rint(out["mo"][0]).astype(np.int32)
        P[b] = out["po"][0].astype(np.float32)
    return M, P
